# revision 1
# baseline (speedup 1.0000x reference)
"""Trainium2 Bass kernel for a DFloat11 Llama decoder layer (8-way tensor parallel).

Layer: RMSNorm -> QKV(+RoPE) -> causal GQA attention -> o_proj -> residual
       -> RMSNorm -> SwiGLU MLP -> residual.
Shapes: B=1, S=2048, H=2048, NH=16 q-heads, NKV=8 kv-heads, HD=128, I=8192.

Sharding (8 NeuronCores, one chip):
  - every core computes RMSNorm(x) for all tokens (replicated, cheap)
  - head-parallel QKV + attention: 2 q-heads + 1 kv-head per core
  - AllToAll flips head-parallel attention output to sequence-parallel
  - o_proj + residual + RMSNorm2 computed per 256-token slice (full o_w per core)
  - AllGather restores the full normed activation for the tensor-parallel MLP
  - gate/up row-sharded, down column-sharded; down partials are ReduceScattered
    in fp32, chunked over H so the collectives overlap MLP compute; each rank's
    RS output rows are exactly its own token slice, so the second residual is
    added locally after the RS
  - host concatenates the 8 [256, 2048] output slices

Matmuls run in bf16 with fp32 PSUM accumulation; softmax, norms, RoPE and both
residual paths stay fp32.  The large weights (o/gate/up/down) are cast
fp32->bf16 into DRAM staging once at kernel start (SWDGE cast DMA), then
streamed through the DMA xbar transpose straight into [contraction, out] SBUF
tiles as each consumer phase needs them.
"""

import sys

if "/opt/trn_rl_repo" not in sys.path:
    sys.path.insert(0, "/opt/trn_rl_repo")

import numpy as np

F32 = None
BF16 = None

NC = 8            # cores
S = 2048          # tokens
H = 2048          # hidden
NH = 16           # q heads
NKV = 8           # kv heads
HD = 128          # head dim
I = 8192          # mlp intermediate
SL = S // NC      # 256 tokens per core
QPC = NH // NC    # 2 q heads per core
ISH = I // NC     # 1024 mlp rows per core
EPS = 1e-6
ISQ = 1.0 / float(np.sqrt(HD))

SPLITS = [4194304, 6291456, 8388608, 12582912, 29360128, 46137344]

NT = S // 128     # 16 token tiles
NHB = H // 128    # 16 hidden blocks
NQ = S // 512     # 4 query tiles of 512
NIB = ISH // 128  # 8 intermediate blocks per core

_PROG = None
_SIM_MODE = False  # replace collectives with local DMAs (for TimelineSim)


def _masks_np():
    k = np.arange(128)[:, None]
    q = np.arange(512)[None, :]
    return [(k + off <= q).astype(np.float32) for off in (0, 128, 256, 384)]


def _build_program():
    import ml_dtypes
    import concourse.bacc as bacc
    import concourse.tile as tile
    import concourse.mybir as mybir

    global F32, BF16
    F32 = mybir.dt.float32
    BF16 = mybir.dt.bfloat16

    ndev = 1 if _SIM_MODE else NC
    nc = bacc.Bacc("TRN2", target_bir_lowering=False, debug=False, num_devices=ndev)

    io = {}

    def inp(name, shape):
        io[name] = nc.dram_tensor(name, shape, F32, kind="ExternalInput").ap()

    inp("hidden", [S, H])          # replicated full hidden states
    inp("hidden_sl", [SL, H])      # this core's residual rows
    inp("qw", [QPC * HD, H])
    inp("kw", [HD, H])
    inp("vw", [HD, H])
    inp("ow", [H, NH * HD])        # full o_proj weight (replicated)
    inp("gw", [ISH, H])
    inp("uw", [ISH, H])
    inp("dw", [H, ISH])            # column shard of down_w
    inp("ilw", [H])
    inp("plw", [H])
    inp("cos", [S, HD])
    inp("sin", [S, HD])
    io["out_slice"] = nc.dram_tensor("out_slice", [SL, H], F32,
                                     kind="ExternalOutput").ap()

    io["masks"] = [
        nc.inline_tensor(m.astype(ml_dtypes.bfloat16), name=f"mask{i}").ap()
        for i, m in enumerate(_masks_np())
    ]
    io["eye"] = nc.inline_tensor(np.eye(128, dtype=np.float32), name="eye128").ap()

    with tile.TileContext(nc) as tc:
        _emit(tc, nc, mybir, io)
    nc.compile()
    return nc


def _emit(tc, nc, mybir, io):
    rg = [list(range(NC))]
    AF = mybir.ActivationFunctionType
    ALU = mybir.AluOpType

    with tc.tile_pool(name="keep", bufs=1) as keep, \
         tc.tile_pool(name="dram", bufs=1, space="DRAM") as dram:

        # ---------------- long-lived tensors ----------------
        h2 = keep.tile([128, 2, H], F32, name="h2")
        ilw_sb = keep.tile([128, NHB], F32, name="ilw_sb")
        nc.sync.dma_start(ilw_sb[:], io["ilw"].rearrange("(j p) -> p j", p=128))
        plw_bc = keep.tile([128, H], F32, name="plw_bc")
        eye = keep.tile([128, 128], F32, name="eye")
        nc.sync.dma_start(eye[:], io["eye"])
        ones_col = keep.tile([128, 1], BF16, name="ones_col")
        nc.vector.memset(ones_col[:], 1.0)
        ones_row = keep.tile([1, 128], BF16, name="ones_row")
        nc.vector.memset(ones_row[:], 1.0)
        ones_row_f = keep.tile([1, 128], F32, name="ones_row_f")
        nc.vector.memset(ones_row_f[:], 1.0)
        eps_t = keep.tile([128, 1], F32, name="eps_t")
        nc.vector.memset(eps_t[:], EPS)
        eps_row = keep.tile([1, 1], F32, name="eps_row")
        nc.vector.memset(eps_row[:], EPS)

        era1_cm = tc.tile_pool(name="era1", bufs=1)
        era1 = era1_cm.__enter__()
        v_sb = era1.tile([128, NT, HD], BF16, name="v_sb")
        qT = era1.tile([128, QPC, S], BF16, name="qT")
        kT = era1.tile([128, S], BF16, name="kT")
        attnT = era1.tile([128, QPC, S], BF16, name="attnT")
        attn_all = era1.tile([128, NH, SL], BF16, name="attn_all")
        mask_sb = era1.tile([128, 4, 512], BF16, name="mask_sb")
        cosT = era1.tile([128, S], F32, name="cosT")
        sinT = era1.tile([128, S], F32, name="sinT")

        # ln-weight broadcast tiles via K=1 fp32 matmuls
        with tc.tile_pool(name="lnb", bufs=2) as lp, \
             tc.tile_pool(name="lnb_ps", bufs=2, space="PSUM") as lps:
            for src, dst in ((io["plw"], plw_bc),):
                row = lp.tile([1, H], F32, name="lnrow", tag="lnrow")
                nc.sync.dma_start(row[:], src.rearrange("(a h) -> a h", a=1))
                for c4 in range(4):
                    ps = lps.tile([128, 512], F32, name="lnps", tag="lnps")
                    nc.tensor.matmul(ps[:], ones_row_f[:],
                                     row[:, c4 * 512:(c4 + 1) * 512],
                                     start=True, stop=True)
                    nc.scalar.copy(dst[:, c4 * 512:(c4 + 1) * 512], ps[:])

        # collective bounce buffers + bf16 weight staging (Local internal DRAM)
        a2a_in = dram.tile([S, SL], BF16, name="a2a_in")
        a2a_out = dram.tile([S, SL], BF16, name="a2a_out")
        ag_in = dram.tile([SL, H], BF16, name="ag_in")
        ag_out = dram.tile([S, H], BF16, name="ag_out")
        rs_in = [dram.tile([S, 512], F32, name=f"rs_in{c}") for c in range(4)]
        rs_out = [dram.tile([SL, 512], F32, name=f"rs_out{c}") for c in range(4)]
        h_st = dram.tile([S, H], BF16, name="h_st")
        o_st = dram.tile([H, NH * HD], BF16, name="o_st")
        g_st = dram.tile([ISH, H], BF16, name="g_st")
        u_st = dram.tile([ISH, H], BF16, name="u_st")
        d_st = dram.tile([H, ISH], BF16, name="d_st")

        # RoPE tables -> [hd, tok] via PE transpose (fp32)
        with tc.tile_pool(name="rope_tr", bufs=3) as rp, \
             tc.tile_pool(name="rope_ps", bufs=3, space="PSUM") as rps:
            for src, dst in ((io["cos"], cosT), (io["sin"], sinT)):
                for ti in range(NT):
                    t_nat = rp.tile([128, 128], F32, name="t_nat", tag="t_nat")
                    nc.sync.dma_start(t_nat[:], src[ti * 128:(ti + 1) * 128, :])
                    t_ps = rps.tile([128, 128], F32, name="t_ps", tag="t_ps")
                    nc.tensor.transpose(t_ps[:], t_nat[:], eye[:])
                    nc.scalar.copy(dst[:, ti * 128:(ti + 1) * 128], t_ps[:])

        # QKV weight strips: cast fp32->bf16 then xbar-transpose
        qkvw_cm = tc.tile_pool(name="qkvw", bufs=1)
        qkvw = qkvw_cm.__enter__()
        with tc.tile_pool(name="wbfp", bufs=2) as pwb:
            def wT_strip(src_rows, tag):
                wbf = pwb.tile([128, H], BF16, name=f"{tag}_bf", tag="wbf")
                nc.gpsimd.dma_start(wbf[:], src_rows)
                wt = qkvw.tile([128, NHB, 128], BF16, name=f"{tag}_t")
                nc.scalar.dma_start(wt[:], wbf[:], transpose=True)
                return wt
            qwt = [wT_strip(io["qw"][m * 128:(m + 1) * 128, :], f"q{m}")
                   for m in range(QPC)]
            kwt = wT_strip(io["kw"], "k")
            vwt = wT_strip(io["vw"], "v")

        # causal masks (needed from attention chunk 0 on)
        for i in range(4):
            nc.sync.dma_start(mask_sb[:, i, :], io["masks"][i])

        def stage_hidden(n):
            for ti in range(n * 4, n * 4 + 4):
                nc.gpsimd.dma_start(h_st[ti * 128:(ti + 1) * 128, :],
                                    io["hidden"][ti * 128:(ti + 1) * 128, :])

        # ---------------- interleaved pipeline: norm1 -> QKV -> attention ---
        xtp_cm = tc.tile_pool(name="xtp", bufs=3)
        xtp = xtp_cm.__enter__()

        n1_cm = tc.tile_pool(name="n1", bufs=2)
        p1 = n1_cm.__enter__()
        prt_cm = tc.tile_pool(name="rope_tmp", bufs=2)
        prt = prt_cm.__enter__()
        pa_cm = tc.tile_pool(name="att", bufs=6)
        pa = pa_cm.__enter__()
        pa2_cm = tc.tile_pool(name="att2", bufs=1)
        pa2 = pa2_cm.__enter__()
        mmps_cm = tc.tile_pool(name="mm_ps", bufs=3, space="PSUM")
        mmps = mmps_cm.__enter__()
        vps_cm = tc.tile_pool(name="v_ps", bufs=1, space="PSUM")
        vps = vps_cm.__enter__()
        dps_cm = tc.tile_pool(name="den_ps", bufs=1, space="PSUM")
        dnps = dps_cm.__enter__()
        nps_cm = tc.tile_pool(name="norm_ps", bufs=1, space="PSUM")
        nnps = nps_cm.__enter__()
        avps_cm = tc.tile_pool(name="av_ps", bufs=2, space="PSUM")
        avps = avps_cm.__enter__()

        def rope_apply(ps, dst_lo, dst_hi, n):
            sl = slice(n * 512, (n + 1) * 512)
            t1 = prt.tile([64, 512], F32, name="t1", tag="ropetmp")
            t2 = prt.tile([64, 512], F32, name="t2", tag="ropetmp")
            nc.vector.tensor_mul(t1[:], ps[0:64, :], cosT[0:64, sl])
            nc.vector.tensor_mul(t2[:], ps[64:128, :], sinT[0:64, sl])
            nc.vector.tensor_sub(dst_lo, t1[:], t2[:])
            t3 = prt.tile([64, 512], F32, name="t3", tag="ropetmp")
            t4 = prt.tile([64, 512], F32, name="t4", tag="ropetmp")
            nc.vector.tensor_mul(t3[:], ps[64:128, :], cosT[64:128, sl])
            nc.vector.tensor_mul(t4[:], ps[0:64, :], sinT[64:128, sl])
            nc.vector.tensor_add(dst_hi, t3[:], t4[:])

        def emit_A(n):
            # raw x.T chunk via DRAM-source xbar transposes, then RMS norm in
            # transposed space: sum(x^2) over the partition (h) axis via
            # ones-matmuls, rstd broadcast back across partitions.
            xT = xtp.tile([128, NHB, 512], BF16, name=f"xT{n}", tag="xT")
            for tl in range(4):
                ti = n * 4 + tl
                nc.scalar.dma_start(xT[:, :, tl * 128:(tl + 1) * 128],
                                    h_st[ti * 128:(ti + 1) * 128, :],
                                    transpose=True)
            nden = nnps.tile([1, 512], F32, name="nden", tag="nden")
            for j in range(NHB):
                x2t = p1.tile([128, 512], BF16, name="x2t", tag="x2t")
                nc.vector.tensor_mul(x2t[:], xT[:, j, :], xT[:, j, :])
                nc.tensor.matmul(nden[:], ones_col[:], x2t[:],
                                 start=(j == 0), stop=(j == NHB - 1))
            std_row = p1.tile([1, 512], F32, name="std_row", tag="srow")
            nc.scalar.activation(std_row[:], nden[:], AF.Sqrt,
                                 bias=eps_row[:], scale=1.0 / H)
            rstd_row = p1.tile([1, 512], F32, name="rstd_row", tag="rrow")
            nc.vector.reciprocal(rstd_row[:], std_row[:])
            rstd_bc = p1.tile([128, 512], F32, name="rstd_bc", tag="rbc")
            nc.gpsimd.partition_broadcast(rstd_bc[:], rstd_row[:])
            for j in range(NHB):
                nc.vector.scalar_tensor_tensor(xT[:, j, :], xT[:, j, :],
                                               ilw_sb[:, j:j + 1], rstd_bc[:],
                                               ALU.mult, ALU.mult)
            return xT

        def emit_B(n, xT):
            sl = slice(n * 512, (n + 1) * 512)
            for m in range(QPC):
                ps = mmps.tile([128, 512], F32, name="ps_q", tag="mm512")
                for j in range(NHB):
                    nc.tensor.matmul(ps[:], qwt[m][:, j, :], xT[:, j, :],
                                     start=(j == 0), stop=(j == NHB - 1))
                rope_apply(ps, qT[0:64, m, sl], qT[64:128, m, sl], n)
            ps = mmps.tile([128, 512], F32, name="ps_k", tag="mm512")
            for j in range(NHB):
                nc.tensor.matmul(ps[:], kwt[:, j, :], xT[:, j, :],
                                 start=(j == 0), stop=(j == NHB - 1))
            rope_apply(ps, kT[0:64, sl], kT[64:128, sl], n)
            psv = vps.tile([128, 512], F32, name="ps_v", tag="v_ps")
            for tl in range(4):
                for j in range(NHB):
                    nc.tensor.matmul(psv[:, tl * 128:(tl + 1) * 128],
                                     xT[:, j, tl * 128:(tl + 1) * 128],
                                     vwt[:, j, :], start=(j == 0),
                                     stop=(j == NHB - 1))
            nc.vector.tensor_copy(
                v_sb[:, n * 4:(n + 1) * 4, :].rearrange("p t d -> p (t d)"),
                psv[:])

        def emit_C(n):
            q0 = n * 512
            for hq in range(QPC):
                kts = list(range(0, 4 * n + 4))
                den = dnps.tile([1, 512], F32, name="den", tag="den")
                av = avps.tile([128, 512], F32, name="av", tag="av")
                for idx, kt in enumerate(kts):
                    k0 = kt * 128
                    s_ps = mmps.tile([128, 512], F32, name="s_ps", tag="mm512")
                    nc.tensor.matmul(s_ps[:], kT[:, k0:k0 + 128],
                                     qT[:, hq, q0:q0 + 512],
                                     start=True, stop=True)
                    es = pa.tile([128, 512], BF16, name="es", tag="es")
                    nc.scalar.activation(es[:], s_ps[:], AF.Exp, scale=ISQ)
                    if k0 > q0 - 128:
                        esm = pa.tile([128, 512], BF16, name="esm", tag="es")
                        nc.vector.tensor_mul(esm[:], es[:],
                                             mask_sb[:, (k0 - q0) // 128, :])
                        es = esm
                    first, last = idx == 0, idx == len(kts) - 1
                    nc.tensor.matmul(den[:], ones_col[:], es[:],
                                     start=first, stop=last)
                    nc.tensor.matmul(av[:], v_sb[:, kt, :], es[:],
                                     start=first, stop=last)
                rec = pa2.tile([1, 512], F32, name="rec", tag="rec")
                nc.vector.reciprocal(rec[:], den[:])
                rb = pa2.tile([128, 512], F32, name="rb", tag="rb")
                nc.gpsimd.partition_broadcast(rb[:], rec[:])
                nc.vector.tensor_mul(attnT[:, hq, q0:q0 + 512], av[:], rb[:])
            for j in (2 * n, 2 * n + 1):
                nc.sync.dma_start(
                    a2a_in[j * SL:(j + 1) * SL, :].rearrange(
                        "(h p) t -> p h t", p=128),
                    attnT[:, :, j * SL:(j + 1) * SL])

        # pipelined emission: A(n+1) overlaps C(n); weight staging casts are
        # emitted inside the pipeline so they fill compute shadows
        stage_hidden(0)
        xts = [emit_A(0)]
        stage_hidden(1)
        xts.append(emit_A(1))
        emit_B(0, xts[0])
        stage = [lambda: nc.gpsimd.dma_start(o_st[:], io["ow"]),
                 lambda: nc.gpsimd.dma_start(g_st[:], io["gw"]),
                 lambda: (nc.gpsimd.dma_start(u_st[:], io["uw"]),
                          nc.gpsimd.dma_start(d_st[:], io["dw"]))]
        for n in range(1, NQ):
            if n + 1 < NQ:
                stage_hidden(n + 1)
                xts.append(emit_A(n + 1))
            emit_C(n - 1)
            stage[n - 1]()
            emit_B(n, xts[n])
        emit_C(NQ - 1)

        for cm in (avps_cm, nps_cm, dps_cm, vps_cm, mmps_cm, pa2_cm, pa_cm,
                   prt_cm, n1_cm, xtp_cm):
            cm.__exit__(None, None, None)

        # head-parallel -> sequence-parallel
        if _SIM_MODE:
            nc.gpsimd.dma_start(a2a_out[:], a2a_in[:])
        else:
            nc.gpsimd.collective_compute("AllToAll", mybir.AluOpType.bypass,
                                         replica_groups=rg,
                                         ins=[a2a_in.opt()], outs=[a2a_out.opt()])
        nc.sync.dma_start(attn_all[:],
                          a2a_out[:].rearrange("(b p) t -> p b t", p=128))

        # ---------------- phase D: o_proj + residual + rmsnorm2 ------------
        # stream o_w.T strips over the contraction (head-row) dim; the two
        # [128, 2048] psum tiles accumulate the full output slice.
        with tc.tile_pool(name="opw", bufs=6) as opw, \
             tc.tile_pool(name="orsd", bufs=1) as orsd, \
             tc.tile_pool(name="o_ps", bufs=1, space="PSUM") as ops:
            resid = orsd.tile([128, 2, H], F32, name="resid", tag="resid")
            nc.sync.dma_start(resid[:],
                              io["hidden_sl"].rearrange("(t p) h -> p t h", p=128))
            pso = [ops.tile([128, H], F32, name=f"ps_o{t}", tag=f"o_ps{t}")
                   for t in range(2)]
            for b in range(NH):
                otb = opw.tile([128, NHB, 128], BF16, name="otb", tag="otb")
                nc.scalar.dma_start(otb[:], o_st[:, b * 128:(b + 1) * 128],
                                    transpose=True)
                for t in range(2):
                    for hc in range(4):
                        nc.tensor.matmul(pso[t][:, hc * 512:(hc + 1) * 512],
                                         attn_all[:, b, t * 128:(t + 1) * 128],
                                         otb[:, hc * 4:(hc + 1) * 4, :],
                                         start=(b == 0), stop=(b == NH - 1))
            for t in range(2):
                nc.vector.tensor_add(h2[:, t, :], pso[t][:], resid[:, t, :])

        with tc.tile_pool(name="n2", bufs=1) as p2:
            for t in range(2):            # rmsnorm2 on the slice
                sq = p2.tile([128, 512], F32, name="sq2", tag="sq2")
                ssum4 = p2.tile([128, 4], F32, name="ssum42", tag="st24")
                for ci in range(4):
                    nc.scalar.activation(sq[:], h2[:, t, ci * 512:(ci + 1) * 512],
                                         AF.Square, accum_out=ssum4[:, ci:ci + 1])
                ssum = p2.tile([128, 1], F32, name="ssum2", tag="st2a")
                nc.vector.tensor_reduce(ssum[:], ssum4[:], mybir.AxisListType.X,
                                        ALU.add)
                std = p2.tile([128, 1], F32, name="std2", tag="st2b")
                nc.scalar.activation(std[:], ssum[:], AF.Sqrt,
                                     bias=eps_t[:], scale=1.0 / H)
                rstd = p2.tile([128, 1], F32, name="rstd2", tag="st2c")
                nc.vector.reciprocal(rstd[:], std[:])
                x2n = p2.tile([128, H], BF16, name="x2n", tag="x2n")
                nc.vector.scalar_tensor_tensor(x2n[:], h2[:, t, :], rstd[:],
                                               plw_bc[:], ALU.mult, ALU.mult)
                nc.sync.dma_start(ag_in[t * 128:(t + 1) * 128, :], x2n[:])

        if _SIM_MODE:
            for r in range(NC):
                nc.gpsimd.dma_start(ag_out[r * SL:(r + 1) * SL, :], ag_in[:])
        else:
            nc.gpsimd.collective_compute("AllGather", mybir.AluOpType.bypass,
                                         replica_groups=rg,
                                         ins=[ag_in.opt()], outs=[ag_out.opt()])

        qkvw_cm.__exit__(None, None, None)
        era1_cm.__exit__(None, None, None)
        era2_cm = tc.tile_pool(name="era2", bufs=1)
        era2 = era2_cm.__enter__()
        x2Tc = [era2.tile([128, NHB, 512], BF16, name=f"x2T{n}")
                for n in range(NQ)]
        actT = era2.tile([128, NIB, S], BF16, name="actT")

        # x2 full -> transposed straight from the DRAM AllGather output;
        # first MLP weight strips interleave right after x2 chunk 0
        guT0 = []
        for ti in range(4):
            nc.scalar.dma_start(x2Tc[0][:, :, ti * 128:ti * 128 + 128],
                                ag_out[ti * 128:(ti + 1) * 128, :],
                                transpose=True)

        # ---------------- phase E: MLP gate/up + silu ----------------
        with tc.tile_pool(name="mlp", bufs=2) as pm, \
             tc.tile_pool(name="g_ps", bufs=3, space="PSUM") as gps, \
             tc.tile_pool(name="u_ps", bufs=3, space="PSUM") as ups:
            gT0 = pm.tile([128, NHB, 128], BF16, name="gT", tag="gT")
            nc.scalar.dma_start(gT0[:], g_st[0:128, :], transpose=True)
            uT0 = pm.tile([128, NHB, 128], BF16, name="uT", tag="uT")
            nc.scalar.dma_start(uT0[:], u_st[0:128, :], transpose=True)
            for ti in range(4, NT):
                nc.scalar.dma_start(x2Tc[ti // 4][:, :, (ti % 4) * 128:
                                                  (ti % 4) * 128 + 128],
                                    ag_out[ti * 128:(ti + 1) * 128, :],
                                    transpose=True)
            for m in range(NIB):
                if m == 0:
                    gT, uT = gT0, uT0
                else:
                    gT = pm.tile([128, NHB, 128], BF16, name="gT", tag="gT")
                    nc.scalar.dma_start(gT[:], g_st[m * 128:(m + 1) * 128, :],
                                        transpose=True)
                    uT = pm.tile([128, NHB, 128], BF16, name="uT", tag="uT")
                    nc.scalar.dma_start(uT[:], u_st[m * 128:(m + 1) * 128, :],
                                        transpose=True)
                for n in range(NQ):
                    sl = slice(n * 512, (n + 1) * 512)
                    psg = gps.tile([128, 512], F32, name="psg", tag="psg")
                    psu = ups.tile([128, 512], F32, name="psu", tag="psu")
                    for j in range(NHB):
                        nc.tensor.matmul(psg[:], gT[:, j, :], x2Tc[n][:, j, :],
                                         start=(j == 0), stop=(j == NHB - 1))
                    for j in range(NHB):
                        nc.tensor.matmul(psu[:], uT[:, j, :], x2Tc[n][:, j, :],
                                         start=(j == 0), stop=(j == NHB - 1))
                    sg = pm.tile([128, 512], F32, name="sg", tag="sg")
                    nc.scalar.activation(sg[:], psg[:], AF.Silu)
                    nc.vector.tensor_mul(actT[:, m, sl], sg[:], psu[:])

        # ---------------- phase F: down_proj + chunked RS + residual -------
        with tc.tile_pool(name="down", bufs=2) as pd, \
             tc.tile_pool(name="d_ps", bufs=2, space="PSUM") as dps, \
             tc.tile_pool(name="d_st", bufs=4) as pst:
            for c in range(4):
                dT = pd.tile([128, NIB, 4, 128], BF16, name="dT", tag="dT")
                for s in range(4):
                    nc.scalar.dma_start(
                        dT[:, :, s, :],
                        d_st[c * 512 + s * 128:c * 512 + (s + 1) * 128, :],
                        transpose=True)
                for t in range(NT):
                    ps = dps.tile([128, 512], F32, name="ps_d", tag="d_ps")
                    for mi in range(NIB):
                        nc.tensor.matmul(ps[:], actT[:, mi, t * 128:(t + 1) * 128],
                                         dT[:, mi, :, :],
                                         start=(mi == 0), stop=(mi == NIB - 1))
                    st = pst.tile([128, 512], F32, name="st", tag="d_st")
                    nc.scalar.copy(st[:], ps[:])
                    nc.sync.dma_start(rs_in[c][t * 128:(t + 1) * 128, :], st[:])
                if _SIM_MODE:
                    nc.gpsimd.dma_start(rs_out[c][:], rs_in[c][0:SL, :])
                else:
                    nc.gpsimd.collective_compute(
                        "ReduceScatter", mybir.AluOpType.add, replica_groups=rg,
                        ins=[rs_in[c].opt()], outs=[rs_out[c].opt()])
                mlp_sl = pst.tile([128, 2, 512], F32, name="mlp_sl", tag="mlp_sl")
                nc.sync.dma_start(
                    mlp_sl[:], rs_out[c][:].rearrange("(t p) h -> p t h", p=128))
                fin = pst.tile([128, 2, 512], F32, name="fin", tag="mlp_sl")
                nc.vector.tensor_add(fin[:], mlp_sl[:],
                                     h2[:, :, c * 512:(c + 1) * 512])
                nc.sync.dma_start(
                    io["out_slice"][:, c * 512:(c + 1) * 512].rearrange(
                        "(t p) h -> p t h", p=128), fin[:])
        era2_cm.__exit__(None, None, None)


# ---------------------------------------------------------------------------
# host wrapper
# ---------------------------------------------------------------------------

def _shard_inputs(hidden_states, flat_weights, input_ln_w, post_ln_w, cos, sin):
    hid = np.ascontiguousarray(hidden_states.reshape(S, H), dtype=np.float32)
    fw = flat_weights
    offs = [0] + SPLITS + [fw.shape[0]]
    q_w = fw[offs[0]:offs[1]].reshape(NH * HD, H)
    k_w = fw[offs[1]:offs[2]].reshape(NKV * HD, H)
    v_w = fw[offs[2]:offs[3]].reshape(NKV * HD, H)
    o_w = np.ascontiguousarray(fw[offs[3]:offs[4]].reshape(H, NH * HD))
    up_w = fw[offs[4]:offs[5]].reshape(I, H)
    gate_w = fw[offs[5]:offs[6]].reshape(I, H)
    down_w = fw[offs[6]:offs[7]].reshape(H, I)

    in_maps = []
    for c in range(NC):
        in_maps.append({
            "hidden": hid,
            "hidden_sl": np.ascontiguousarray(hid[c * SL:(c + 1) * SL]),
            "qw": np.ascontiguousarray(q_w[c * QPC * HD:(c + 1) * QPC * HD]),
            "kw": np.ascontiguousarray(k_w[c * HD:(c + 1) * HD]),
            "vw": np.ascontiguousarray(v_w[c * HD:(c + 1) * HD]),
            "ow": o_w,
            "gw": np.ascontiguousarray(gate_w[c * ISH:(c + 1) * ISH]),
            "uw": np.ascontiguousarray(up_w[c * ISH:(c + 1) * ISH]),
            "dw": np.ascontiguousarray(down_w[:, c * ISH:(c + 1) * ISH]),
            "ilw": np.ascontiguousarray(input_ln_w, dtype=np.float32),
            "plw": np.ascontiguousarray(post_ln_w, dtype=np.float32),
            "cos": np.ascontiguousarray(cos, dtype=np.float32),
            "sin": np.ascontiguousarray(sin, dtype=np.float32),
        })
    return in_maps


def _get_program():
    global _PROG
    if _PROG is None:
        _PROG = _build_program()
    return _PROG


def run_spmd(in_maps, trace=False):
    import time
    from concourse import bass_utils
    nc = _get_program()
    last = None
    for attempt in range(3):
        try:
            return bass_utils.run_bass_kernel_spmd(
                nc, in_maps, core_ids=list(range(NC)), trace=trace)
        except Exception as e:  # transient NRT_EXEC_UNIT_UNRECOVERABLE wedges
            last = e
            if attempt < 2:
                time.sleep(45)
    raise last


def kernel(hidden_states, flat_weights, input_ln_w, post_ln_w, cos, sin):
    in_maps = _shard_inputs(np.asarray(hidden_states), np.asarray(flat_weights),
                            np.asarray(input_ln_w), np.asarray(post_ln_w),
                            np.asarray(cos), np.asarray(sin))
    res = run_spmd(in_maps)
    out = np.concatenate([res.results[c]["out_slice"] for c in range(NC)], axis=0)
    return out.reshape(1, S, H).astype(np.float32)


def build_sim_program():
    """Single-core, collective-free build for TimelineSim profiling."""
    global _SIM_MODE
    _SIM_MODE = True
    try:
        return _build_program()
    finally:
        _SIM_MODE = False



# revision 2
# speedup vs baseline: 1.4199x; 1.4199x over previous
"""Trainium2 Bass kernel for a DFloat11 Llama decoder layer (8-way TP), v2.

vs v1: all big GEMMs (QKV, gate/up, down) run as fp8e4m3 DoubleRow matmuls
with a hi/lo split on BOTH operands (W ~ Wh+Wl, x ~ xh+xl, products
Wh*xh + Wh*xl + Wl*xh accumulated in one PSUM group) -> 0.75x bf16 PE time
with better-than-bf16 accuracy.  Weights are sharded, layernorm-folded,
transposed, scaled by WS=64 and hi/lo fp8-split on the host; the device
reads them directly (no staging casts).  RMSNorms are computed on each
core's own 256-token slice in natural space; the normalized activations are
hi/lo split and AllGathered (x1 for QKV, x2 for the MLP).  Attention and
o_proj stay bf16 (A2A flips head-parallel attention to sequence-parallel
o_proj as in v1).  The down_proj partials ReduceScatter in bf16, chunked
over H to overlap with compute.
"""

import sys

if "/opt/trn_rl_repo" not in sys.path:
    sys.path.insert(0, "/opt/trn_rl_repo")

import numpy as np

NC = 8
S = 2048
H = 2048
NH = 16
NKV = 8
HD = 128
I = 8192
SL = S // NC        # 256
QPC = NH // NC      # 2
ISH = I // NC       # 1024
EPS = 1e-6
ISQ = 1.0 / float(np.sqrt(HD))
WS = 64.0

SPLITS = [4194304, 6291456, 8388608, 12582912, 29360128, 46137344]

NT = S // 128       # 16
NHB = H // 128      # 16
NQ = S // 512       # 4
NIB = ISH // 128    # 8
NJP = NHB // 2      # 8 k-tile pairs over H
NMP = NIB // 2      # 4 k-tile pairs over ISH

_PROG = None
_SIM_MODE = False


def _masks_np():
    k = np.arange(128)[:, None]
    q = np.arange(512)[None, :]
    return [(k + off <= q).astype(np.float32) for off in (0, 128, 256, 384)]


def _build_program():
    import ml_dtypes
    import concourse.bacc as bacc
    import concourse.tile as tile
    import concourse.mybir as mybir

    F32 = mybir.dt.float32
    BF16 = mybir.dt.bfloat16
    F8 = mybir.dt.float8e4
    DR = mybir.MatmulPerfMode.DoubleRow

    ndev = 1 if _SIM_MODE else NC
    nc = bacc.Bacc("TRN2", target_bir_lowering=False, debug=False,
                   num_devices=ndev)

    io = {}

    def inp(name, shape, dt):
        io[name] = nc.dram_tensor(name, shape, dt, kind="ExternalInput").ap()

    inp("resid", [SL, H], F32)            # this core's hidden rows
    inp("qwh", [128, NHB, QPC * HD], F8)  # q rows (ilw folded, *WS), hi
    inp("qwl", [128, NHB, QPC * HD], F8)
    inp("kwh", [128, NHB, HD], F8)
    inp("kwl", [128, NHB, HD], F8)
    inp("vwh", [128, NHB, HD], F8)
    inp("vwl", [128, NHB, HD], F8)
    inp("owt", [128, NH, H], BF16)        # o_w.T tiled [d'%128, d'//128, h]
    inp("gwh", [128, NHB, ISH], F8)       # gate rows (plw folded, *WS)
    inp("gwl", [128, NHB, ISH], F8)
    inp("uwh", [128, NHB, ISH], F8)
    inp("uwl", [128, NHB, ISH], F8)
    inp("dwh", [128, NIB, H], F8)         # down.T shard [i%128, i//128, h]
    inp("dwl", [128, NIB, H], F8)
    inp("cosT", [128, S], F32)            # rope tables transposed, / WS
    inp("sinT", [128, S], F32)
    io["out_slice"] = nc.dram_tensor("out_slice", [SL, H], F32,
                                     kind="ExternalOutput").ap()

    io["masks"] = [
        nc.inline_tensor(m.astype(ml_dtypes.bfloat16), name=f"mask{i}").ap()
        for i, m in enumerate(_masks_np())
    ]

    with tile.TileContext(nc) as tc:
        _emit(tc, nc, mybir, io, F32, BF16, F8, DR)
    nc.compile()
    return nc


def _emit(tc, nc, mybir, io, F32, BF16, F8, DR):
    rg = [list(range(NC))]
    AF = mybir.ActivationFunctionType
    ALU = mybir.AluOpType

    def dr3(ps, ah, al, bh, bl, pairs, asl, bsl):
        """3-product hi/lo DoubleRow accumulation into psum ps.
        asl(t, jp) / bsl(t, jp) slice tensor t at k-tile pair jp."""
        first = True
        for a, b, last_p in ((ah, bh, False), (ah, bl, False), (al, bh, True)):
            for jp in range(pairs):
                nc.tensor.matmul(ps, asl(a, jp), bsl(b, jp),
                                 start=first,
                                 stop=(last_p and jp == pairs - 1),
                                 perf_mode=DR)
                first = False

    with tc.tile_pool(name="keep", bufs=1) as keep, \
         tc.tile_pool(name="dram", bufs=1, space="DRAM") as dram:

        h2 = keep.tile([128, 2, H], F32, name="h2")
        resid = keep.tile([128, 2, H], F32, name="resid")
        nc.sync.dma_start(resid[:],
                          io["resid"].rearrange("(t p) h -> p t h", p=128))
        eps_t = keep.tile([128, 1], F32, name="eps_t")
        nc.vector.memset(eps_t[:], EPS)
        ones_col = keep.tile([128, 1], BF16, name="ones_col")
        nc.vector.memset(ones_col[:], 1.0)

        # ---- weights / tables resident in SBUF ----
        qkv_cm = tc.tile_pool(name="qkvw", bufs=1)
        qkvw = qkv_cm.__enter__()
        wsb = {}
        for nm, cols in (("qwh", QPC * HD), ("qwl", QPC * HD), ("kwh", HD),
                         ("kwl", HD), ("vwh", HD), ("vwl", HD)):
            wsb[nm] = qkvw.tile([128, NHB, cols], F8, name=nm)
            nc.gpsimd.dma_start(wsb[nm][:], io[nm])
        era1_cm = tc.tile_pool(name="era1", bufs=1)
        era1 = era1_cm.__enter__()
        cosT = era1.tile([128, S], F32, name="cosT")
        sinT = era1.tile([128, S], F32, name="sinT")
        nc.sync.dma_start(cosT[:], io["cosT"])
        nc.sync.dma_start(sinT[:], io["sinT"])
        mask_sb = era1.tile([128, 4, 512], BF16, name="mask_sb")
        for i in range(4):
            nc.sync.dma_start(mask_sb[:, i, :], io["masks"][i])

        v_sb = era1.tile([128, NT, HD], BF16, name="v_sb")
        qT = era1.tile([128, QPC, S], BF16, name="qT")
        kT = era1.tile([128, S], BF16, name="kT")
        attnT = era1.tile([128, QPC, S], BF16, name="attnT")
        attn_all = era1.tile([128, NH, SL], BF16, name="attn_all")

        # ---- collective bounce buffers (internal DRAM) ----
        ag1h_in = dram.tile([NHB * 128, SL], F8, name="ag1h_in")
        ag1l_in = dram.tile([NHB * 128, SL], F8, name="ag1l_in")
        ag1h_out = dram.tile([NC * NHB * 128, SL], F8, name="ag1h_out")
        ag1l_out = dram.tile([NC * NHB * 128, SL], F8, name="ag1l_out")
        ag2h_in = dram.tile([NHB * 128, SL], F8, name="ag2h_in")
        ag2l_in = dram.tile([NHB * 128, SL], F8, name="ag2l_in")
        ag2h_out = dram.tile([NC * NHB * 128, SL], F8, name="ag2h_out")
        ag2l_out = dram.tile([NC * NHB * 128, SL], F8, name="ag2l_out")
        a2a_in = dram.tile([S, SL], BF16, name="a2a_in")
        a2a_out = dram.tile([S, SL], BF16, name="a2a_out")
        rs_in = [dram.tile([S, 512], BF16, name=f"rs_in{c}") for c in range(4)]
        rs_out = [dram.tile([SL, 512], BF16, name=f"rs_out{c}")
                  for c in range(4)]

        NR = NHB * 128
        def allgather(ag_in, ag_out):
            if _SIM_MODE:
                for r in range(NC):
                    nc.gpsimd.dma_start(ag_out[r * NR:(r + 1) * NR, :],
                                        ag_in[:])
            else:
                nc.gpsimd.collective_compute(
                    "AllGather", mybir.AluOpType.bypass, replica_groups=rg,
                    ins=[ag_in.opt()], outs=[ag_out.opt()])

        # ---- norm + hi/lo split + transpose of a [128,2,H] f32 slice ----
        def norm_split(src, agh, agl, tag):
            with tc.tile_pool(name=f"ns_{tag}", bufs=1) as pn:
                acc = pn.tile([128, 2], F32, name="acc", tag="nacc")
                for u in range(2):
                    sq = pn.tile([128, H], F32, name="sq", tag=f"nsq{u}")
                    nc.scalar.activation(sq[:], src[:, u, :], AF.Square,
                                         accum_out=acc[:, u:u + 1])
                std = pn.tile([128, 2], F32, name="std", tag="nstd")
                nc.scalar.activation(std[:], acc[:], AF.Sqrt,
                                     bias=eps_t[:], scale=1.0 / H)
                rstd = pn.tile([128, 2], F32, name="rstd", tag="nrstd")
                nc.vector.reciprocal(rstd[:], std[:])
                xn = pn.tile([128, 2, H], BF16, name="xn", tag="nxn")
                for u in range(2):
                    nc.vector.tensor_scalar_mul(xn[:, u, :], src[:, u, :],
                                                rstd[:, u:u + 1])
                xnT = pn.tile([128, NHB, SL], BF16, name="xnT", tag="nxnT")
                for u in range(2):
                    nc.scalar.dma_start(xnT[:, :, u * 128:(u + 1) * 128],
                                        xn[:, u, :], transpose=True)
                xh = pn.tile([128, NHB, SL], F8, name="xh", tag="nxh")
                nc.scalar.activation(
                    xh[:].rearrange("p a b -> p (a b)"),
                    xnT[:].rearrange("p a b -> p (a b)"), AF.Copy)
                dif = pn.tile([128, NHB, SL], F32, name="dif", tag="ndif")
                nc.vector.tensor_sub(
                    dif[:].rearrange("p a b -> p (a b)"),
                    xnT[:].rearrange("p a b -> p (a b)"),
                    xh[:].rearrange("p a b -> p (a b)"))
                xl = pn.tile([128, NHB, SL], F8, name="xl", tag="nxl")
                nc.scalar.activation(
                    xl[:].rearrange("p a b -> p (a b)"),
                    dif[:].rearrange("p a b -> p (a b)"), AF.Copy)
                nc.sync.dma_start(
                    agh[:].rearrange("(j p) t -> p j t", p=128), xh[:])
                nc.sync.dma_start(
                    agl[:].rearrange("(j p) t -> p j t", p=128), xl[:])

        norm_split(resid, ag1h_in, ag1l_in, "n1")
        allgather(ag1h_in, ag1h_out)
        allgather(ag1l_in, ag1l_out)

        # ---- phase B: QKV (fp8 DR) + rope;  phase C: attention (bf16) ----
        ow_cm = tc.tile_pool(name="oww", bufs=6)
        oww = ow_cm.__enter__()
        xp_cm = tc.tile_pool(name="xp", bufs=4)
        xp = xp_cm.__enter__()
        prt_cm = tc.tile_pool(name="rope_tmp", bufs=2)
        prt = prt_cm.__enter__()
        pa_cm = tc.tile_pool(name="att", bufs=6)
        pa = pa_cm.__enter__()
        pa2_cm = tc.tile_pool(name="att2", bufs=1)
        pa2 = pa2_cm.__enter__()
        mmps_cm = tc.tile_pool(name="mm_ps", bufs=3, space="PSUM")
        mmps = mmps_cm.__enter__()
        vps_cm = tc.tile_pool(name="v_ps", bufs=1, space="PSUM")
        vps = vps_cm.__enter__()
        dps_cm = tc.tile_pool(name="den_ps", bufs=1, space="PSUM")
        dnps = dps_cm.__enter__()
        avps_cm = tc.tile_pool(name="av_ps", bufs=2, space="PSUM")
        avps = avps_cm.__enter__()

        def rope_apply(ps, dst_lo, dst_hi, n):
            sl = slice(n * 512, (n + 1) * 512)
            t1 = prt.tile([64, 512], F32, name="t1", tag="ropetmp")
            t2 = prt.tile([64, 512], F32, name="t2", tag="ropetmp")
            nc.vector.tensor_mul(t1[:], ps[0:64, :], cosT[0:64, sl])
            nc.vector.tensor_mul(t2[:], ps[64:128, :], sinT[0:64, sl])
            nc.vector.tensor_sub(dst_lo, t1[:], t2[:])
            t3 = prt.tile([64, 512], F32, name="t3", tag="ropetmp")
            t4 = prt.tile([64, 512], F32, name="t4", tag="ropetmp")
            nc.vector.tensor_mul(t3[:], ps[64:128, :], cosT[64:128, sl])
            nc.vector.tensor_mul(t4[:], ps[0:64, :], sinT[64:128, sl])
            nc.vector.tensor_add(dst_hi, t3[:], t4[:])

        ag1h_v = ag1h_out[:].rearrange("(r j p) t -> p j r t", r=NC, j=NHB)
        ag1l_v = ag1l_out[:].rearrange("(r j p) t -> p j r t", r=NC, j=NHB)

        def emit_B(n):
            # x chunk tiles [128, NHB, 512]
            xh = xp.tile([128, NHB, 512], F8, name=f"xh{n}", tag="xck")
            xl = xp.tile([128, NHB, 512], F8, name=f"xl{n}", tag="xck")
            for r in range(2):
                idx = 2 * n + r
                nc.gpsimd.dma_start(
                    xh[:, :, r * SL:(r + 1) * SL],
                    ag1h_v[:, :, idx:idx + 1, :].rearrange(
                        "p j r t -> p j (r t)"))
                nc.gpsimd.dma_start(
                    xl[:, :, r * SL:(r + 1) * SL],
                    ag1l_v[:, :, idx:idx + 1, :].rearrange(
                        "p j r t -> p j (r t)"))

            def xsl(t, jp):
                return t[:, 2 * jp:2 * jp + 2, :]

            for m in range(QPC):
                ps = mmps.tile([128, 512], F32, name="ps_q", tag="mm512")
                wsl = lambda t, jp: t[:, 2 * jp:2 * jp + 2,
                                      m * 128:(m + 1) * 128]
                dr3(ps[:], wsb["qwh"], wsb["qwl"], xh, xl, NJP, wsl, xsl)
                rope_apply(ps, qT[0:64, m, n * 512:(n + 1) * 512],
                           qT[64:128, m, n * 512:(n + 1) * 512], n)
            ps = mmps.tile([128, 512], F32, name="ps_k", tag="mm512")
            wsl = lambda t, jp: t[:, 2 * jp:2 * jp + 2, :]
            dr3(ps[:], wsb["kwh"], wsb["kwl"], xh, xl, NJP, wsl, xsl)
            rope_apply(ps, kT[0:64, n * 512:(n + 1) * 512],
                       kT[64:128, n * 512:(n + 1) * 512], n)
            psv = vps.tile([128, 512], F32, name="ps_v", tag="v_ps")
            for tl in range(4):
                xvsl = lambda t, jp: t[:, 2 * jp:2 * jp + 2,
                                       tl * 128:(tl + 1) * 128]
                vwsl = lambda t, jp: t[:, 2 * jp:2 * jp + 2, :]
                dr3(psv[:, tl * 128:(tl + 1) * 128], xh, xl,
                    wsb["vwh"], wsb["vwl"], NJP, xvsl, vwsl)
            nc.scalar.activation(
                v_sb[:, n * 4:(n + 1) * 4, :].rearrange("p t d -> p (t d)"),
                psv[:], AF.Copy, scale=1.0 / WS)

        def emit_C(n):
            q0 = n * 512
            for hq in range(QPC):
                kts = list(range(0, 4 * n + 4))
                den = dnps.tile([1, 512], F32, name="den", tag="den")
                av = avps.tile([128, 512], F32, name="av", tag="av")
                for idx, kt in enumerate(kts):
                    k0 = kt * 128
                    s_ps = mmps.tile([128, 512], F32, name="s_ps", tag="mm512")
                    nc.tensor.matmul(s_ps[:], kT[:, k0:k0 + 128],
                                     qT[:, hq, q0:q0 + 512],
                                     start=True, stop=True)
                    es = pa.tile([128, 512], BF16, name="es", tag="es")
                    nc.scalar.activation(es[:], s_ps[:], AF.Exp, scale=ISQ)
                    if k0 > q0 - 128:
                        esm = pa.tile([128, 512], BF16, name="esm", tag="es")
                        nc.vector.tensor_mul(esm[:], es[:],
                                             mask_sb[:, (k0 - q0) // 128, :])
                        es = esm
                    first, last = idx == 0, idx == len(kts) - 1
                    nc.tensor.matmul(den[:], ones_col[:], es[:],
                                     start=first, stop=last)
                    nc.tensor.matmul(av[:], v_sb[:, kt, :], es[:],
                                     start=first, stop=last)
                rec = pa2.tile([1, 512], F32, name="rec", tag="rec")
                nc.vector.reciprocal(rec[:], den[:])
                rb = pa2.tile([128, 512], F32, name="rb", tag="rb")
                nc.gpsimd.partition_broadcast(rb[:], rec[:])
                nc.vector.tensor_mul(attnT[:, hq, q0:q0 + 512], av[:], rb[:])
            for j in (2 * n, 2 * n + 1):
                nc.sync.dma_start(
                    a2a_in[j * SL:(j + 1) * SL, :].rearrange(
                        "(h p) t -> p h t", p=128),
                    attnT[:, :, j * SL:(j + 1) * SL])

        emit_B(0)
        emit_B(1)
        emit_C(0)
        emit_B(2)
        emit_C(1)
        emit_B(3)
        emit_C(2)
        emit_C(3)

        # head-parallel -> sequence-parallel
        if _SIM_MODE:
            nc.gpsimd.dma_start(a2a_out[:], a2a_in[:])
        else:
            nc.gpsimd.collective_compute("AllToAll", mybir.AluOpType.bypass,
                                         replica_groups=rg,
                                         ins=[a2a_in.opt()],
                                         outs=[a2a_out.opt()])
        nc.sync.dma_start(attn_all[:],
                          a2a_out[:].rearrange("(b p) t -> p b t", p=128))

        for cm in (avps_cm, dps_cm, vps_cm, pa2_cm, pa_cm, prt_cm, mmps_cm,
                   xp_cm):
            cm.__exit__(None, None, None)

        # ---- phase D: o_proj + residual + norm2 + split + AG2 ----
        with tc.tile_pool(name="o_ps", bufs=1, space="PSUM") as ops:
            pso = [ops.tile([128, H], F32, name=f"ps_o{t}", tag=f"o_ps{t}")
                   for t in range(2)]
            for b in range(NH):
                otb = oww.tile([128, H], BF16, name=f"ow{b}", tag="owt")
                nc.gpsimd.dma_start(otb[:], io["owt"][:, b, :])
                for t in range(2):
                    for hc in range(4):
                        nc.tensor.matmul(pso[t][:, hc * 512:(hc + 1) * 512],
                                         attn_all[:, b, t * 128:(t + 1) * 128],
                                         otb[:, hc * 512:(hc + 1) * 512],
                                         start=(b == 0), stop=(b == NH - 1))
            for t in range(2):
                nc.vector.tensor_add(h2[:, t, :], pso[t][:], resid[:, t, :])

        norm_split(h2, ag2h_in, ag2l_in, "n2")
        allgather(ag2h_in, ag2h_out)
        allgather(ag2l_in, ag2l_out)

        ow_cm.__exit__(None, None, None)
        era1_cm.__exit__(None, None, None)
        qkv_cm.__exit__(None, None, None)
        # ---- phase E: MLP gate/up (fp8 DR) ----
        era2_cm = tc.tile_pool(name="era2", bufs=1)
        era2 = era2_cm.__enter__()
        px2_cm = tc.tile_pool(name="px2", bufs=1)
        px2 = px2_cm.__enter__()
        x2h = [px2.tile([128, NHB, 512], F8, name=f"x2h{n}")
               for n in range(NQ)]
        x2l = [px2.tile([128, NHB, 512], F8, name=f"x2l{n}")
               for n in range(NQ)]
        ag2h_v = ag2h_out[:].rearrange("(r j p) t -> p j r t", r=NC, j=NHB)
        ag2l_v = ag2l_out[:].rearrange("(r j p) t -> p j r t", r=NC, j=NHB)
        for n in range(NQ):
            for r in range(2):
                idx = 2 * n + r
                nc.gpsimd.dma_start(
                    x2h[n][:, :, r * SL:(r + 1) * SL],
                    ag2h_v[:, :, idx:idx + 1, :].rearrange(
                        "p j r t -> p j (r t)"))
                nc.gpsimd.dma_start(
                    x2l[n][:, :, r * SL:(r + 1) * SL],
                    ag2l_v[:, :, idx:idx + 1, :].rearrange(
                        "p j r t -> p j (r t)"))
        actH = era2.tile([128, NIB, S], F8, name="actH")
        actL = era2.tile([128, NIB, S], F8, name="actL")
        dwh = era2.tile([128, NIB, H], F8, name="dwh")
        dwl = era2.tile([128, NIB, H], F8, name="dwl")
        nc.gpsimd.dma_start(dwh[:], io["dwh"])
        nc.gpsimd.dma_start(dwl[:], io["dwl"])

        with tc.tile_pool(name="mlp", bufs=3) as pm, \
             tc.tile_pool(name="g_ps", bufs=3, space="PSUM") as gps, \
             tc.tile_pool(name="u_ps", bufs=3, space="PSUM") as ups:
            for m in range(NIB):
                gh = pm.tile([128, NHB, 128], F8, name="gh", tag="gh")
                gl = pm.tile([128, NHB, 128], F8, name="gl", tag="gl")
                uh = pm.tile([128, NHB, 128], F8, name="uh", tag="uh")
                ul = pm.tile([128, NHB, 128], F8, name="ul", tag="ul")
                msl = slice(m * 128, (m + 1) * 128)
                nc.gpsimd.dma_start(gh[:], io["gwh"][:, :, msl])
                nc.gpsimd.dma_start(gl[:], io["gwl"][:, :, msl])
                nc.gpsimd.dma_start(uh[:], io["uwh"][:, :, msl])
                nc.gpsimd.dma_start(ul[:], io["uwl"][:, :, msl])
                for n in range(NQ):
                    sl = slice(n * 512, (n + 1) * 512)
                    psg = gps.tile([128, 512], F32, name="psg", tag="psg")
                    psu = ups.tile([128, 512], F32, name="psu", tag="psu")
                    wsl = lambda t, jp: t[:, 2 * jp:2 * jp + 2, :]
                    xhsl = lambda t, jp: t[:, 2 * jp:2 * jp + 2, :]
                    dr3(psg[:], gh, gl, x2h[n], x2l[n], NJP, wsl, xhsl)
                    dr3(psu[:], uh, ul, x2h[n], x2l[n], NJP, wsl, xhsl)
                    sg = pm.tile([128, 512], F32, name="sg", tag="sg")
                    nc.scalar.activation(sg[:], psg[:], AF.Silu,
                                         scale=1.0 / WS)
                    a = pm.tile([128, 512], F32, name="a", tag="a")
                    nc.vector.scalar_tensor_tensor(a[:], psu[:], 1.0 / WS,
                                                   sg[:], ALU.mult, ALU.mult)
                    nc.scalar.activation(actH[:, m, sl], a[:], AF.Copy)
                    dif = pm.tile([128, 512], F32, name="dif", tag="dif")
                    nc.vector.tensor_sub(dif[:], a[:], actH[:, m, sl])
                    nc.scalar.activation(actL[:, m, sl], dif[:], AF.Copy)

        px2_cm.__exit__(None, None, None)

        # ---- phase F: down (fp8 DR) + chunked bf16 RS + residual ----
        with tc.tile_pool(name="d_ps", bufs=2, space="PSUM") as dps, \
             tc.tile_pool(name="d_st", bufs=3) as pst:
            for c in range(4):
                csl = slice(c * 512, (c + 1) * 512)
                for tg in range(4):
                    stb = pst.tile([128, 4, 512], BF16, name="stb", tag="stb")
                    for ti in range(4):
                        t = tg * 4 + ti
                        ps = dps.tile([128, 512], F32, name="ps_d", tag="d_ps")
                        asl = lambda tt, mp: tt[:, 2 * mp:2 * mp + 2,
                                                t * 128:(t + 1) * 128]
                        dsl = lambda tt, mp: tt[:, 2 * mp:2 * mp + 2, csl]
                        dr3(ps[:], actH, actL, dwh, dwl, NMP, asl, dsl)
                        nc.scalar.activation(stb[:, ti, :], ps[:], AF.Copy,
                                             scale=1.0 / WS)
                    nc.sync.dma_start(
                        rs_in[c][tg * 512:(tg + 1) * 512, :].rearrange(
                            "(t p) h -> p t h", p=128), stb[:])
                if _SIM_MODE:
                    nc.gpsimd.dma_start(rs_out[c][:], rs_in[c][0:SL, :])
                else:
                    nc.gpsimd.collective_compute(
                        "ReduceScatter", mybir.AluOpType.add,
                        replica_groups=rg,
                        ins=[rs_in[c].opt()], outs=[rs_out[c].opt()])
                mlp_sl = pst.tile([128, 2, 512], BF16, name="mlp_sl",
                                  tag="mlp_sl")
                nc.sync.dma_start(
                    mlp_sl[:],
                    rs_out[c][:].rearrange("(t p) h -> p t h", p=128))
                fin = pst.tile([128, 2, 512], F32, name="fin", tag="fin")
                nc.vector.tensor_add(fin[:], mlp_sl[:], h2[:, :, csl])
                nc.sync.dma_start(
                    io["out_slice"][:, csl].rearrange(
                        "(t p) h -> p t h", p=128), fin[:])
        era2_cm.__exit__(None, None, None)


# ---------------------------------------------------------------------------
# host wrapper
# ---------------------------------------------------------------------------

def _wtile(w, scale):
    """[R, H_contract] row-major weight -> hi/lo fp8 tiles
    [128, H_contract//128, R] (contraction on partitions)."""
    import ml_dtypes
    F8 = ml_dtypes.float8_e4m3
    wT = np.ascontiguousarray(w.T.astype(np.float32) * scale)  # [K, R]
    K, R = wT.shape
    arr = wT.reshape(K // 128, 128, R).transpose(1, 0, 2)      # [128, KT, R]
    hi = arr.astype(F8)
    lo = (arr - hi.astype(np.float32)).astype(F8)
    return np.ascontiguousarray(hi), np.ascontiguousarray(lo)


def _shard_inputs(hidden_states, flat_weights, input_ln_w, post_ln_w, cos,
                  sin):
    import ml_dtypes
    BF = ml_dtypes.bfloat16
    hid = np.ascontiguousarray(hidden_states.reshape(S, H), dtype=np.float32)
    fw = np.asarray(flat_weights, dtype=np.float32)
    offs = [0] + SPLITS + [fw.shape[0]]
    q_w = fw[offs[0]:offs[1]].reshape(NH * HD, H)
    k_w = fw[offs[1]:offs[2]].reshape(NKV * HD, H)
    v_w = fw[offs[2]:offs[3]].reshape(NKV * HD, H)
    o_w = fw[offs[3]:offs[4]].reshape(H, NH * HD)
    up_w = fw[offs[4]:offs[5]].reshape(I, H)
    gate_w = fw[offs[5]:offs[6]].reshape(I, H)
    down_w = fw[offs[6]:offs[7]].reshape(H, I)
    ilw = np.asarray(input_ln_w, np.float32)[None, :]
    plw = np.asarray(post_ln_w, np.float32)[None, :]

    # o_w.T tiled [128, NH, H] bf16
    owt = np.ascontiguousarray(
        o_w.T.reshape(NH, HD, H).transpose(1, 0, 2).astype(BF))
    # rope tables transposed, premultiplied by 1/WS
    cosT = np.ascontiguousarray(np.asarray(cos, np.float32).T) / WS
    sinT = np.ascontiguousarray(np.asarray(sin, np.float32).T) / WS

    in_maps = []
    for c in range(NC):
        qwh, qwl = _wtile(q_w[c * QPC * HD:(c + 1) * QPC * HD] * ilw, WS)
        kwh, kwl = _wtile(k_w[c * HD:(c + 1) * HD] * ilw, WS)
        vwh, vwl = _wtile(v_w[c * HD:(c + 1) * HD] * ilw, WS)
        gwh, gwl = _wtile(gate_w[c * ISH:(c + 1) * ISH] * plw, WS)
        uwh, uwl = _wtile(up_w[c * ISH:(c + 1) * ISH] * plw, WS)
        # down: shard columns, contraction = local i
        dsh = down_w[:, c * ISH:(c + 1) * ISH]          # [H, ISH]
        dwh, dwl = _wtile(dsh, WS)                      # [128, NIB, H]
        in_maps.append({
            "resid": np.ascontiguousarray(hid[c * SL:(c + 1) * SL]),
            "qwh": qwh, "qwl": qwl, "kwh": kwh, "kwl": kwl,
            "vwh": vwh, "vwl": vwl,
            "owt": owt, "gwh": gwh, "gwl": gwl, "uwh": uwh, "uwl": uwl,
            "dwh": dwh, "dwl": dwl,
            "cosT": np.ascontiguousarray(cosT),
            "sinT": np.ascontiguousarray(sinT),
        })
    return in_maps


def _get_program():
    global _PROG
    if _PROG is None:
        _PROG = _build_program()
    return _PROG


def run_spmd(in_maps, trace=False):
    import time
    from concourse import bass_utils
    nc = _get_program()
    last = None
    for attempt in range(3):
        try:
            return bass_utils.run_bass_kernel_spmd(
                nc, in_maps, core_ids=list(range(NC)), trace=trace)
        except Exception as e:
            last = e
            if attempt < 2:
                time.sleep(45)
    raise last


def kernel(hidden_states, flat_weights, input_ln_w, post_ln_w, cos, sin):
    in_maps = _shard_inputs(np.asarray(hidden_states),
                            np.asarray(flat_weights),
                            np.asarray(input_ln_w), np.asarray(post_ln_w),
                            np.asarray(cos), np.asarray(sin))
    res = run_spmd(in_maps)
    out = np.concatenate([res.results[c]["out_slice"] for c in range(NC)],
                         axis=0)
    return out.reshape(1, S, H).astype(np.float32)


def build_sim_program():
    global _SIM_MODE
    _SIM_MODE = True
    try:
        return _build_program()
    finally:
        _SIM_MODE = False
